# revision 14
# baseline (speedup 1.0000x reference)
"""ColBERT MaxSim kernel for 8 Trainium2 NeuronCores (Bass/Tile).

Strategy (v3): data-parallel over the 256-doc batch (32 docs per core).

Key optimizations over the v1 kernel (75.9us):
- Valid-token compaction on host: the doc mask keeps ~50% of tokens, so
  each doc's valid tokens are gathered into a fixed LV-slot prefix
  (LV = max valid count rounded up to 32; 288 for the spec data) and
  zero-padded.  Cuts HBM traffic and PE/ACT/DVE streaming ~44%.
- Normalize scores instead of vectors: sim is computed on unnormalized
  projections; the per-token 1/|d| scale is applied to the [32q, LV]
  score rows of 4 docs packed in one PSUM bank, so the rsqrt runs once
  per 4 docs instead of on every [128, LV] projection tile.
- The per-token sumsq ones-matmul packs both docs of an even pair into
  one fp8 DoubleRow matmul (zeroed cross-interleave weights give each
  doc its own 32-partition output group; DR can only target partition
  base 0, so odd pairs use per-doc 32-col ones matmuls).
- ss/sim matmuls are emitted one pair behind the projection so the PE
  never stalls on the ACT square / DVE copy of the same pair.
- The first two doc pairs are emitted BEFORE the query stage (and the
  projection PSUM pool sits in different banks than the query pool),
  so doc projections start as soon as the first doc DMA lands instead
  of serializing behind the whole query pipeline.

Per core:
  q_proj = Wt.T @ qT                [128dim, 128q]  -> l2norm -> q_norm
  per pair p (16):
    pd    = Wt8.T @ slab[p]         [128dim, 2, LV] (fp8 DoubleRow, 6 MMs)
    sq    = pd^2                    (ACT, fp8 out)
    dnr   = pd                      (DVE copy, bf16 out)
    ssq[quad]  += ones.T @ sq       (fp8 DR matmul / 32-col matmuls)
    simq[quad] += q32.T @ dnr       (2 bf16 matmuls -> 32 rows each)
  per quad g (8):
    inv  = rsqrt(ssq + eps)         (ACT)
    maxcol[:, g] = max_t(simq*inv)  (DVE mul + reduce_max)
  out[4, 8] = blockones.T @ maxcol  (sum over queries)
"""

import numpy as np
import ml_dtypes

import concourse.bass as bass
import concourse.bass_isa as bass_isa
import concourse.bacc as bacc
import concourse.mybir as mybir
import concourse.tile as tile
from concourse.bass_utils import run_bass_kernel_spmd

N_CORES = 8
H, HC, P = 768, 6, 128   # hidden dim, h-chunks, partitions
LD = 512                 # doc tokens (uncompacted)
DIM = 128                # projection dim
DPC = 32                 # docs per core
QPC = 128                # query vectors per core (4 batches x 32)
PPQ = 8                  # passages per query
NS = 4                   # doc slabs per core
SPD = 8                  # docs per slab
BF16 = mybir.dt.bfloat16
FP8 = mybir.dt.float8e4
F32 = mybir.dt.float32
EPS2 = 1e-12

# pack both docs of an even pair into one fp8 DoubleRow sumsq matmul
USE_DR_SS = True
# fp8 DoubleRow for the sim matmul too (q_norm + projections in fp8)
USE_DR_SIM = False
# fuse normalize-multiply + token max into one DVE tensor_tensor_reduce
# (disabled: crashes the device at runtime despite passing CoreSim)
USE_TTR = False
# pairs of slack between a projection and its ss/sim consumers
DELAY = 1

_NC_CACHE = {}
_CUR_LV = [LD]


def _rsqrt_act(nc, out, in_, bias_ap):
    """out = 1/sqrt(in_ + bias). Emits the Rsqrt activation directly
    (bass's helper refuses it; the 40k-entry reciprocal_sqrt HW table is
    plenty accurate for this kernel's fp8-dominated error budget)."""
    eng = nc.scalar
    ins = [eng.lower_ap(in_), eng.lower_ap(bias_ap),
           mybir.ImmediateValue(dtype=mybir.dt.float32, value=1.0),
           mybir.ImmediateValue(dtype=mybir.dt.float32, value=0.0)]
    return eng.add_instruction(mybir.InstActivation(
        name=nc.get_next_instruction_name(),
        func=mybir.ActivationFunctionType.Rsqrt,
        ins=ins, outs=[eng.lower_ap(out)]))


def _build_nc(LV):
    nc = bacc.Bacc()
    dt_d = nc.declare_dram_parameter("dt", [NS, P, SPD, HC, LV], FP8,
                                     isOutput=False)
    qt_d = nc.declare_dram_parameter("qt", [P, HC, QPC], BF16, isOutput=False)
    wt_d = nc.declare_dram_parameter("wt", [P, HC, DIM], BF16, isOutput=False)
    wt8_d = nc.declare_dram_parameter("wt8", [P, HC, DIM], FP8, isOutput=False)
    out_d = nc.declare_dram_parameter("out", [4, DPC // 4], F32, isOutput=True)

    SQ_DT = FP8 if USE_DR_SS else BF16
    DNR_DT = FP8 if USE_DR_SIM else BF16

    with tile.TileContext(nc) as tc:
        with tc.tile_pool(name="const", bufs=1) as const:
            # first DMAs: what pair-0's projection needs (wt8 + docs 0-1)
            wt8_s = const.tile([P, HC, DIM], FP8)
            nc.sync.dma_start(out=wt8_s, in_=wt8_d[:])

            # Matmul (LDWEIGHTS) instructions only support a single sync
            # wait, so every matmul operand must be produced by a single
            # engine: constants are staged through ACT copies so PE waits
            # coalesce onto one semaphore.
            jones_raw = const.tile([P, P], BF16)
            nc.vector.memset(jones_raw, 1.0)
            jones = const.tile([P, P], BF16)   # all-ones lhsT [K=128, M=128]
            nc.scalar.copy(jones, jones_raw)
            if USE_DR_SS:
                # DR sumsq lhsT [K=128, 2, 64]: cols 0:32 pick interleave 0
                # (doc j=0), cols 32:64 pick interleave 1 (doc j=1)
                odr_raw = const.tile([P, 2, 64], BF16)
                nc.vector.memset(odr_raw, 0.0)
                nc.vector.memset(odr_raw[:, 0, 0:32], 1.0)
                nc.vector.memset(odr_raw[:, 1, 32:64], 1.0)
                ones_dr = const.tile([P, 2, 64], FP8)
                nc.scalar.copy(ones_dr, odr_raw)
                jones8 = const.tile([P, 32], FP8)   # fp8 ones for odd pairs
                nc.scalar.copy(jones8, jones_raw[:, :32])
            blk_raw = const.tile([P, 4], F32)   # block-diag ones: col b = ones
            nc.vector.memset(blk_raw, 0.0)      # on partitions 32b..32b+32
            for b in range(4):
                nc.vector.memset(blk_raw[32 * b:32 * b + 32, b:b + 1], 1.0)
            blockones = const.tile([P, 4], F32)
            nc.scalar.copy(blockones, blk_raw)
            eps_t = const.tile([P, 1], F32)     # rsqrt bias (l2norm eps^2)
            nc.vector.memset(eps_t, EPS2)
            maxcol = const.tile([P, DPC // 4], F32)  # [4docs x 32q, quad-col]
            q_norm = const.tile([DIM, QPC], BF16)
            dummy = const.tile([P, 1], BF16)    # TTR discard target

            wt_s = const.tile([P, HC, DIM], BF16)
            nc.sync.dma_start(out=wt_s, in_=wt_d[:])
            qt_s = const.tile([P, HC, QPC], BF16)
            nc.sync.dma_start(out=qt_s, in_=qt_d[:])

            with (
                tc.tile_pool(name="slabp", bufs=3) as slabp,
                tc.tile_pool(name="work", bufs=3) as work,
                tc.tile_pool(name="psumP", bufs=2,
                             space=bass.MemorySpace.PSUM) as psumP,
            ):
                state = {"slab": None}

                def do_pair_front(pair):
                    """slab DMA + projection + square/copy for one pair."""
                    if pair % (SPD // 2) == 0:
                        s = pair // (SPD // 2)
                        slab = slabp.tile([P, SPD, HC, LV], FP8, tag="slab")
                        state["slab"] = slab
                        if s == 0:
                            # staggered fill so PE can start sooner
                            nc.sync.dma_start(out=slab[:, 0:2],
                                              in_=dt_d[0, :, 0:2])
                            nc.sync.dma_start(out=slab[:, 2:4],
                                              in_=dt_d[0, :, 2:4])
                            nc.sync.dma_start(out=slab[:, 4:8],
                                              in_=dt_d[0, :, 4:8])
                        else:
                            nc.sync.dma_start(out=slab, in_=dt_d[s])
                    slab = state["slab"]
                    r2 = (pair % (SPD // 2)) * 2   # doc-in-slab of doc j=0

                    # projection (fp8 DoubleRow: 256-deep contraction/pass).
                    # doc-major order + per-doc square/cast halves so each
                    # doc's epilogue inputs are ready as early as possible
                    # (the pair-batched versions put the whole ACT square on
                    # the PSUM-rotation critical path).
                    pd = psumP.tile([P, 2, LD], F32, tag="pd")
                    sq = work.tile([P, 2, LV], SQ_DT, tag="sq", bufs=DELAY + 2)
                    dnr = work.tile([P, 2, LV], DNR_DT, tag="dnr",
                                    bufs=DELAY + 2)
                    for j in range(2):
                        for c in range(0, HC, 2):
                            nc.tensor.matmul(
                                pd[:, j, :LV], wt8_s[:, c:c + 2, :],
                                slab[:, r2 + j, c:c + 2, :],
                                start=(c == 0), stop=(c == HC - 2),
                                perf_mode=mybir.MatmulPerfMode.DoubleRow)
                        nc.scalar.square(sq[:, j, :], pd[:, j, :LV])
                        nc.vector.tensor_copy(dnr[:, j, :], pd[:, j, :LV])
                    return (pair, sq, dnr)

                # front-run two pairs so doc projections start as soon as
                # the first doc DMA lands (the query stage runs in parallel)
                pending = [do_pair_front(0), do_pair_front(1)]

                # ---- query projection + L2 normalize ----
                with tc.tile_pool(name="qpsum", bufs=1,
                                  space=bass.MemorySpace.PSUM) as qpsum:
                    psq = qpsum.tile([DIM, QPC], F32, tag="pq")
                    for c in range(HC):
                        nc.tensor.matmul(psq, wt_s[:, c, :], qt_s[:, c, :],
                                         start=(c == 0), stop=(c == HC - 1))
                    sqq = const.tile([DIM, QPC], BF16)
                    nc.scalar.square(sqq, psq)
                    ssqb = qpsum.tile([DIM, QPC], F32, tag="ssq")
                    nc.tensor.matmul(ssqb, jones, sqq, start=True, stop=True)
                    invqb = const.tile([DIM, QPC], F32)
                    _rsqrt_act(nc, invqb, ssqb, eps_t[:, :])
                    nc.vector.tensor_mul(q_norm, psq, invqb)
                    if USE_DR_SIM:
                        # DR sim lhsT per q-batch b: cols 0:32 = q_norm(b) on
                        # interleave 0, cols 32:64 = q_norm(b) on interleave 1
                        qdr_raw = const.tile([P, 4, 2, 64], BF16)
                        nc.vector.memset(qdr_raw, 0.0)
                        for b in range(4):
                            nc.vector.tensor_copy(
                                qdr_raw[:, b, 0, 0:32],
                                q_norm[:, 32 * b:32 * b + 32])
                            nc.vector.tensor_copy(
                                qdr_raw[:, b, 1, 32:64],
                                q_norm[:, 32 * b:32 * b + 32])
                        q_dr = const.tile([P, 4, 2, 64], FP8)
                        nc.scalar.copy(q_dr, qdr_raw)

                # ---- doc loop ----
                with (
                    tc.tile_pool(name="psumS", bufs=2,
                                 space=bass.MemorySpace.PSUM) as psumS,
                    tc.tile_pool(name="psumM", bufs=2,
                                 space=bass.MemorySpace.PSUM) as psumM,
                ):
                    quads = {}     # quad idx -> (ssq, simq)

                    def emit_ss_sim(pair, sq, dnr):
                        """ss/sim matmuls (+ quad epilogue) for a projected
                        pair; emitted DELAY pairs late so the PE never
                        stalls on the same pair's ACT square / DVE copy."""
                        if pair % 2 == 0:
                            quads[pair // 2] = (
                                psumS.tile([P, LD], F32, tag="ss",
                                           name="ssq"),
                                psumM.tile([P, LD], F32, tag="sim",
                                           name="simq"))
                        ssq, simq = quads[pair // 2]
                        d0 = 2 * pair
                        po_base = 32 * (d0 % 4)    # 0 (even pair), 64 (odd)
                        qb = d0 // PPQ             # q-batch (same for a pair)
                        # per-token sumsq -> ssq[po_base : po_base+64]
                        if USE_DR_SS and po_base == 0:
                            nc.tensor.matmul(
                                ssq[po_base:po_base + 64, :LV], ones_dr, sq,
                                start=True, stop=True,
                                perf_mode=mybir.MatmulPerfMode.DoubleRow,
                                tile_position=(0, po_base))
                        else:
                            ssl = jones8 if USE_DR_SS else jones[:, :32]
                            for j in range(2):
                                pb = po_base + 32 * j
                                nc.tensor.matmul(ssq[pb:pb + 32, :LV],
                                                 ssl, sq[:, j, :],
                                                 start=True, stop=True,
                                                 tile_position=(0, pb))
                        # raw sim scores -> simq[po_base : po_base+64]
                        if USE_DR_SIM and po_base == 0:
                            nc.tensor.matmul(
                                simq[po_base:po_base + 64, :LV], q_dr[:, qb],
                                dnr, start=True, stop=True,
                                perf_mode=mybir.MatmulPerfMode.DoubleRow,
                                tile_position=(0, po_base))
                        else:
                            for j in range(2):
                                pb = po_base + 32 * j
                                nc.tensor.matmul(
                                    simq[pb:pb + 32, :LV],
                                    q_norm[:, 32 * qb:32 * qb + 32],
                                    dnr[:, j, :], start=True, stop=True,
                                    tile_position=(0, pb))

                        if pair % 2 == 1:
                            g = pair // 2
                            inv = work.tile([P, LV], F32, tag="inv")
                            _rsqrt_act(nc, inv, ssq[:, :LV], eps_t[:, :])
                            # maxcol[:, g] = max_t(simq * inv)
                            if USE_TTR:
                                nc.vector.tensor_tensor_reduce(
                                    dummy.broadcast_to((P, LV)),
                                    simq[:, :LV], inv,
                                    scale=1.0, scalar=-3.0e38,
                                    op0=mybir.AluOpType.mult,
                                    op1=mybir.AluOpType.max,
                                    accum_out=maxcol[:, g:g + 1])
                            else:
                                simn = work.tile([P, LV], BF16, tag="simn")
                                nc.vector.tensor_mul(simn, simq[:, :LV], inv)
                                nc.vector.reduce_max(out=maxcol[:, g:g + 1],
                                                     in_=simn,
                                                     axis=mybir.AxisListType.X)

                    for pair in range(2, DPC // 2):
                        pending.append(do_pair_front(pair))
                        while len(pending) > DELAY:
                            emit_ss_sim(*pending.pop(0))
                    for args in pending:
                        emit_ss_sim(*args)

                    po = psumS.tile([4, DPC // 4], F32, tag="ss")
                    nc.tensor.matmul(po, blockones, maxcol,
                                     start=True, stop=True)
                    out_s = work.tile([4, DPC // 4], F32, tag="outrow")
                    nc.vector.tensor_copy(out_s, po)
                    nc.sync.dma_start(out=out_d[:], in_=out_s)
    nc.compile()
    return nc


def _get_nc(LV):
    if LV not in _NC_CACHE:
        _NC_CACHE[LV] = _build_nc(LV)
    return _NC_CACHE[LV]


def _prep_in_maps(q_hidden, d_hidden, W, d_mask):
    bf16 = ml_dtypes.bfloat16
    fp8 = ml_dtypes.float8_e4m3

    # Valid-token compaction: gather each doc's unmasked tokens into a
    # fixed LV-slot prefix; padding slots gather zeroed (masked) rows.
    cnt = d_mask.sum(1)
    LV = int(min(LD, max(32, ((int(cnt.max()) + 31) // 32) * 32)))
    _CUR_LV[0] = LV
    dh = d_hidden.astype(fp8)
    dh[~d_mask] = 0
    order = np.argsort(~d_mask, axis=1, kind="stable")   # valid tokens first
    dhc = np.take_along_axis(dh, order[:, :LV, None], axis=1)  # [Bd, LV, H]

    wt_t = np.ascontiguousarray(W.T.reshape(HC, P, DIM).transpose(1, 0, 2))
    wt = wt_t.astype(bf16)
    wt8 = wt_t.astype(fp8)
    in_maps = []
    for c in range(N_CORES):
        dsl = dhc[c * DPC:(c + 1) * DPC]          # [32, LV, 768]
        arr = dsl.reshape(NS, SPD, LV, HC, P)     # (s, j, t, c, p)
        dt = np.ascontiguousarray(arr.transpose(0, 4, 1, 3, 2))
        qsl = q_hidden[c * (DPC // PPQ):(c + 1) * (DPC // PPQ)]
        qm = qsl.reshape(QPC, H).T.reshape(HC, P, QPC)    # [6, 128, 128]
        qt = np.ascontiguousarray(qm.transpose(1, 0, 2)).astype(bf16)
        in_maps.append({"dt": dt, "qt": qt, "wt": wt, "wt8": wt8})
    return in_maps


def _run(in_maps, trace=False, **kw):
    res = run_bass_kernel_spmd(
        _get_nc(_CUR_LV[0]), in_maps, core_ids=list(range(N_CORES)),
        trace=trace, **kw)
    # per-core output is [4, DPC//4] with doc = 4*col + row
    out = np.concatenate(
        [res.results[i]["out"].T.reshape(-1) for i in range(N_CORES)])
    return out.astype(np.float32), res


def kernel(q_hidden, d_hidden, W, d_mask, ppq):
    q_hidden = np.asarray(q_hidden, dtype=np.float32)
    d_hidden = np.asarray(d_hidden, dtype=np.float32)
    W = np.asarray(W, dtype=np.float32)
    d_mask = np.asarray(d_mask).astype(bool)
    in_maps = _prep_in_maps(q_hidden, d_hidden, W, d_mask)
    out, _ = _run(in_maps, trace=False)
    return out


# revision 16
# speedup vs baseline: 1.0693x; 1.0693x over previous
"""ColBERT MaxSim kernel for 8 Trainium2 NeuronCores (Bass/Tile).

Strategy (v3): data-parallel over the 256-doc batch (32 docs per core).

Key optimizations over the v1 kernel (75.9us):
- Valid-token compaction on host: the doc mask keeps ~50% of tokens, so
  each doc's valid tokens are gathered into a fixed LV-slot prefix
  (LV = max valid count rounded up to 32; 288 for the spec data) and
  zero-padded.  Cuts HBM traffic and PE/ACT/DVE streaming ~44%.
- Normalize scores instead of vectors: sim is computed on unnormalized
  projections; the per-token 1/|d| scale is applied to the [32q, LV]
  score rows of 4 docs packed in one PSUM bank, so the rsqrt runs once
  per 4 docs instead of on every [128, LV] projection tile.
- The per-token sumsq ones-matmul packs both docs of an even pair into
  one fp8 DoubleRow matmul (zeroed cross-interleave weights give each
  doc its own 32-partition output group; DR can only target partition
  base 0, so odd pairs use per-doc 32-col ones matmuls).
- ss/sim matmuls are emitted one pair behind the projection so the PE
  never stalls on the ACT square / DVE copy of the same pair.
- The first two doc pairs are emitted BEFORE the query stage (and the
  projection PSUM pool sits in different banks than the query pool),
  so doc projections start as soon as the first doc DMA lands instead
  of serializing behind the whole query pipeline.

Per core:
  q_proj = Wt.T @ qT                [128dim, 128q]  -> l2norm -> q_norm
  per pair p (16):
    pd    = Wt8.T @ slab[p]         [128dim, 2, LV] (fp8 DoubleRow, 6 MMs)
    sq    = pd^2                    (ACT, fp8 out)
    dnr   = pd                      (DVE copy, bf16 out)
    ssq[quad]  += ones.T @ sq       (fp8 DR matmul / 32-col matmuls)
    simq[quad] += q32.T @ dnr       (2 bf16 matmuls -> 32 rows each)
  per quad g (8):
    inv  = rsqrt(ssq + eps)         (ACT)
    maxcol[:, g] = max_t(simq*inv)  (DVE mul + reduce_max)
  out[4, 8] = blockones.T @ maxcol  (sum over queries)
"""

import numpy as np
import ml_dtypes

import concourse.bass as bass
import concourse.bass_isa as bass_isa
import concourse.bacc as bacc
import concourse.mybir as mybir
import concourse.tile as tile
from concourse.bass_utils import run_bass_kernel_spmd

N_CORES = 8
H, HC, P = 768, 6, 128   # hidden dim, h-chunks, partitions
LD = 512                 # doc tokens (uncompacted)
DIM = 128                # projection dim
DPC = 32                 # docs per core
QPC = 128                # query vectors per core (4 batches x 32)
PPQ = 8                  # passages per query
NS = 4                   # doc slabs per core
SPD = 8                  # docs per slab
BF16 = mybir.dt.bfloat16
FP8 = mybir.dt.float8e4
F32 = mybir.dt.float32
EPS2 = 1e-12

# pack both docs of an even pair into one fp8 DoubleRow sumsq matmul
USE_DR_SS = True
# fp8 DoubleRow for the sim matmul too (q_norm + projections in fp8)
USE_DR_SIM = False
# fuse normalize-multiply + token max into one DVE tensor_tensor_reduce
# (disabled: crashes the device at runtime despite passing CoreSim)
USE_TTR = False
# pairs of slack between a projection and its ss/sim consumers
DELAY = 1

_NC_CACHE = {}
_CUR_LV = [LD]


def _rsqrt_act(nc, out, in_, bias_ap):
    """out = 1/sqrt(in_ + bias). Emits the Rsqrt activation directly
    (bass's helper refuses it; the 40k-entry reciprocal_sqrt HW table is
    plenty accurate for this kernel's fp8-dominated error budget)."""
    eng = nc.scalar
    ins = [eng.lower_ap(in_), eng.lower_ap(bias_ap),
           mybir.ImmediateValue(dtype=mybir.dt.float32, value=1.0),
           mybir.ImmediateValue(dtype=mybir.dt.float32, value=0.0)]
    return eng.add_instruction(mybir.InstActivation(
        name=nc.get_next_instruction_name(),
        func=mybir.ActivationFunctionType.Rsqrt,
        ins=ins, outs=[eng.lower_ap(out)]))


def _build_nc(LV):
    nc = bacc.Bacc()
    dt_d = nc.declare_dram_parameter("dt", [NS, P, SPD, HC, LV], FP8,
                                     isOutput=False)
    qt_d = nc.declare_dram_parameter("qt", [P, HC, QPC], BF16, isOutput=False)
    wt_d = nc.declare_dram_parameter("wt", [P, HC, DIM], BF16, isOutput=False)
    wt8_d = nc.declare_dram_parameter("wt8", [P, HC, DIM], FP8, isOutput=False)
    out_d = nc.declare_dram_parameter("out", [4, DPC // 4], F32, isOutput=True)

    SQ_DT = FP8 if USE_DR_SS else BF16
    DNR_DT = FP8 if USE_DR_SIM else BF16

    with tile.TileContext(nc) as tc:
        with tc.tile_pool(name="const", bufs=1) as const:
            # first DMAs: what pair-0's projection needs (wt8 + docs 0-1)
            wt8_s = const.tile([P, HC, DIM], FP8)
            nc.sync.dma_start(out=wt8_s, in_=wt8_d[:])

            # Matmul (LDWEIGHTS) instructions only support a single sync
            # wait, so every matmul operand must be produced by a single
            # engine: constants are staged through ACT copies so PE waits
            # coalesce onto one semaphore.
            jones_raw = const.tile([P, P], BF16)
            nc.vector.memset(jones_raw, 1.0)
            jones = const.tile([P, P], BF16)   # all-ones lhsT [K=128, M=128]
            nc.scalar.copy(jones, jones_raw)
            if USE_DR_SS:
                # DR sumsq lhsT [K=128, 2, 64]: cols 0:32 pick interleave 0
                # (doc j=0), cols 32:64 pick interleave 1 (doc j=1)
                odr_raw = const.tile([P, 2, 64], BF16)
                nc.vector.memset(odr_raw, 0.0)
                nc.vector.memset(odr_raw[:, 0, 0:32], 1.0)
                nc.vector.memset(odr_raw[:, 1, 32:64], 1.0)
                ones_dr = const.tile([P, 2, 64], FP8)
                nc.scalar.copy(ones_dr, odr_raw)
                jones8 = const.tile([P, 32], FP8)   # fp8 ones for odd pairs
                nc.scalar.copy(jones8, jones_raw[:, :32])
            blk_raw = const.tile([P, 4], F32)   # block-diag ones: col b = ones
            nc.vector.memset(blk_raw, 0.0)      # on partitions 32b..32b+32
            for b in range(4):
                nc.vector.memset(blk_raw[32 * b:32 * b + 32, b:b + 1], 1.0)
            blockones = const.tile([P, 4], F32)
            nc.scalar.copy(blockones, blk_raw)
            eps_t = const.tile([P, 1], F32)     # rsqrt bias (l2norm eps^2)
            nc.vector.memset(eps_t, EPS2)
            maxcol = const.tile([P, DPC // 4], F32)  # [4docs x 32q, quad-col]
            q_norm = const.tile([DIM, QPC], BF16)
            dummy = const.tile([P, 1], BF16)    # TTR discard target

            wt_s = const.tile([P, HC, DIM], BF16)
            qt_s = const.tile([P, HC, QPC], BF16)

            with (
                tc.tile_pool(name="slabp", bufs=3) as slabp,
                tc.tile_pool(name="work", bufs=3) as work,
                tc.tile_pool(name="psumP", bufs=2,
                             space=bass.MemorySpace.PSUM) as psumP,
            ):
                state = {"slab": None}

                def do_pair_front(pair):
                    """slab DMA + projection + square/copy for one pair."""
                    if pair == 0:
                        # slab 0 fills are staggered (and interleaved with
                        # the wt/qt issues) so pair-0's docs land first and
                        # the PE starts as early as possible
                        slab = slabp.tile([P, SPD, HC, LV], FP8, tag="slab")
                        state["slab"] = slab
                        nc.sync.dma_start(out=slab[:, 0:2], in_=dt_d[0, :, 0:2])
                    elif pair == 1:
                        nc.sync.dma_start(out=state["slab"][:, 2:4],
                                          in_=dt_d[0, :, 2:4])
                    elif pair == 2:
                        nc.sync.dma_start(out=state["slab"][:, 4:8],
                                          in_=dt_d[0, :, 4:8])
                    elif pair % (SPD // 2) == 0:
                        s = pair // (SPD // 2)
                        slab = slabp.tile([P, SPD, HC, LV], FP8, tag="slab")
                        state["slab"] = slab
                        nc.sync.dma_start(out=slab, in_=dt_d[s])
                    slab = state["slab"]
                    r2 = (pair % (SPD // 2)) * 2   # doc-in-slab of doc j=0

                    # projection (fp8 DoubleRow: 256-deep contraction/pass).
                    # doc-major order + per-doc square/cast halves so each
                    # doc's epilogue inputs are ready as early as possible
                    # (the pair-batched versions put the whole ACT square on
                    # the PSUM-rotation critical path).
                    pd = psumP.tile([P, 2, LD], F32, tag="pd")
                    for c in range(0, HC, 2):
                        for j in range(2):
                            nc.tensor.matmul(
                                pd[:, j, :LV], wt8_s[:, c:c + 2, :],
                                slab[:, r2 + j, c:c + 2, :],
                                start=(c == 0), stop=(c == HC - 2),
                                perf_mode=mybir.MatmulPerfMode.DoubleRow)
                    sq = work.tile([P, 2, LV], SQ_DT, tag="sq", bufs=DELAY + 2)
                    nc.scalar.square(sq, pd[:, :, :LV])
                    dnr = work.tile([P, 2, LV], DNR_DT, tag="dnr",
                                    bufs=DELAY + 2)
                    nc.vector.tensor_copy(dnr, pd[:, :, :LV])
                    return (pair, sq, dnr)

                # front-run two pairs so doc projections start as soon as
                # the first doc DMA lands (the query stage runs in parallel)
                pending = [do_pair_front(0)]
                nc.sync.dma_start(out=wt_s, in_=wt_d[:])
                nc.sync.dma_start(out=qt_s, in_=qt_d[:])
                pending.append(do_pair_front(1))

                # ---- query projection + L2 normalize ----
                with tc.tile_pool(name="qpsum", bufs=1,
                                  space=bass.MemorySpace.PSUM) as qpsum:
                    psq = qpsum.tile([DIM, QPC], F32, tag="pq")
                    for c in range(HC):
                        nc.tensor.matmul(psq, wt_s[:, c, :], qt_s[:, c, :],
                                         start=(c == 0), stop=(c == HC - 1))
                    sqq = const.tile([DIM, QPC], BF16)
                    nc.scalar.square(sqq, psq)
                    ssqb = qpsum.tile([DIM, QPC], F32, tag="ssq")
                    nc.tensor.matmul(ssqb, jones, sqq, start=True, stop=True)
                    invqb = const.tile([DIM, QPC], F32)
                    _rsqrt_act(nc, invqb, ssqb, eps_t[:, :])
                    nc.vector.tensor_mul(q_norm, psq, invqb)
                    if USE_DR_SIM:
                        # DR sim lhsT per q-batch b: cols 0:32 = q_norm(b) on
                        # interleave 0, cols 32:64 = q_norm(b) on interleave 1
                        qdr_raw = const.tile([P, 4, 2, 64], BF16)
                        nc.vector.memset(qdr_raw, 0.0)
                        for b in range(4):
                            nc.vector.tensor_copy(
                                qdr_raw[:, b, 0, 0:32],
                                q_norm[:, 32 * b:32 * b + 32])
                            nc.vector.tensor_copy(
                                qdr_raw[:, b, 1, 32:64],
                                q_norm[:, 32 * b:32 * b + 32])
                        q_dr = const.tile([P, 4, 2, 64], FP8)
                        nc.scalar.copy(q_dr, qdr_raw)

                # ---- doc loop ----
                with (
                    tc.tile_pool(name="psumS", bufs=2,
                                 space=bass.MemorySpace.PSUM) as psumS,
                    tc.tile_pool(name="psumM", bufs=2,
                                 space=bass.MemorySpace.PSUM) as psumM,
                ):
                    quads = {}     # quad idx -> (ssq, simq)

                    def emit_ss_sim(pair, sq, dnr):
                        """ss/sim matmuls (+ quad epilogue) for a projected
                        pair; emitted DELAY pairs late so the PE never
                        stalls on the same pair's ACT square / DVE copy."""
                        if pair % 2 == 0:
                            quads[pair // 2] = (
                                psumS.tile([P, LD], F32, tag="ss",
                                           name="ssq"),
                                psumM.tile([P, LD], F32, tag="sim",
                                           name="simq"))
                        ssq, simq = quads[pair // 2]
                        d0 = 2 * pair
                        po_base = 32 * (d0 % 4)    # 0 (even pair), 64 (odd)
                        qb = d0 // PPQ             # q-batch (same for a pair)
                        # per-token sumsq -> ssq[po_base : po_base+64]
                        if USE_DR_SS and po_base == 0:
                            nc.tensor.matmul(
                                ssq[po_base:po_base + 64, :LV], ones_dr, sq,
                                start=True, stop=True,
                                perf_mode=mybir.MatmulPerfMode.DoubleRow,
                                tile_position=(0, po_base))
                        else:
                            ssl = jones8 if USE_DR_SS else jones[:, :32]
                            for j in range(2):
                                pb = po_base + 32 * j
                                nc.tensor.matmul(ssq[pb:pb + 32, :LV],
                                                 ssl, sq[:, j, :],
                                                 start=True, stop=True,
                                                 tile_position=(0, pb))
                        # raw sim scores -> simq[po_base : po_base+64]
                        if USE_DR_SIM and po_base == 0:
                            nc.tensor.matmul(
                                simq[po_base:po_base + 64, :LV], q_dr[:, qb],
                                dnr, start=True, stop=True,
                                perf_mode=mybir.MatmulPerfMode.DoubleRow,
                                tile_position=(0, po_base))
                        else:
                            for j in range(2):
                                pb = po_base + 32 * j
                                nc.tensor.matmul(
                                    simq[pb:pb + 32, :LV],
                                    q_norm[:, 32 * qb:32 * qb + 32],
                                    dnr[:, j, :], start=True, stop=True,
                                    tile_position=(0, pb))

                        if pair % 2 == 1:
                            g = pair // 2
                            inv = work.tile([P, LV], F32, tag="inv")
                            _rsqrt_act(nc, inv, ssq[:, :LV], eps_t[:, :])
                            # maxcol[:, g] = max_t(simq * inv)
                            if USE_TTR:
                                nc.vector.tensor_tensor_reduce(
                                    dummy.broadcast_to((P, LV)),
                                    simq[:, :LV], inv,
                                    scale=1.0, scalar=-3.0e38,
                                    op0=mybir.AluOpType.mult,
                                    op1=mybir.AluOpType.max,
                                    accum_out=maxcol[:, g:g + 1])
                            else:
                                simn = work.tile([P, LV], BF16, tag="simn")
                                nc.vector.tensor_mul(simn, simq[:, :LV], inv)
                                nc.vector.reduce_max(out=maxcol[:, g:g + 1],
                                                     in_=simn,
                                                     axis=mybir.AxisListType.X)

                    for pair in range(2, DPC // 2):
                        pending.append(do_pair_front(pair))
                        while len(pending) > DELAY:
                            emit_ss_sim(*pending.pop(0))
                    for args in pending:
                        emit_ss_sim(*args)

                    po = psumS.tile([4, DPC // 4], F32, tag="ss")
                    nc.tensor.matmul(po, blockones, maxcol,
                                     start=True, stop=True)
                    out_s = work.tile([4, DPC // 4], F32, tag="outrow")
                    nc.vector.tensor_copy(out_s, po)
                    nc.sync.dma_start(out=out_d[:], in_=out_s)
    nc.compile()
    return nc


def _get_nc(LV):
    if LV not in _NC_CACHE:
        _NC_CACHE[LV] = _build_nc(LV)
    return _NC_CACHE[LV]


def _prep_in_maps(q_hidden, d_hidden, W, d_mask):
    bf16 = ml_dtypes.bfloat16
    fp8 = ml_dtypes.float8_e4m3

    # Valid-token compaction: gather each doc's unmasked tokens into a
    # fixed LV-slot prefix; padding slots gather zeroed (masked) rows.
    cnt = d_mask.sum(1)
    LV = int(min(LD, max(32, ((int(cnt.max()) + 31) // 32) * 32)))
    _CUR_LV[0] = LV
    dh = d_hidden.astype(fp8)
    dh[~d_mask] = 0
    order = np.argsort(~d_mask, axis=1, kind="stable")   # valid tokens first
    dhc = np.take_along_axis(dh, order[:, :LV, None], axis=1)  # [Bd, LV, H]

    wt_t = np.ascontiguousarray(W.T.reshape(HC, P, DIM).transpose(1, 0, 2))
    wt = wt_t.astype(bf16)
    wt8 = wt_t.astype(fp8)
    in_maps = []
    for c in range(N_CORES):
        dsl = dhc[c * DPC:(c + 1) * DPC]          # [32, LV, 768]
        arr = dsl.reshape(NS, SPD, LV, HC, P)     # (s, j, t, c, p)
        dt = np.ascontiguousarray(arr.transpose(0, 4, 1, 3, 2))
        qsl = q_hidden[c * (DPC // PPQ):(c + 1) * (DPC // PPQ)]
        qm = qsl.reshape(QPC, H).T.reshape(HC, P, QPC)    # [6, 128, 128]
        qt = np.ascontiguousarray(qm.transpose(1, 0, 2)).astype(bf16)
        in_maps.append({"dt": dt, "qt": qt, "wt": wt, "wt8": wt8})
    return in_maps


def _run(in_maps, trace=False, **kw):
    res = run_bass_kernel_spmd(
        _get_nc(_CUR_LV[0]), in_maps, core_ids=list(range(N_CORES)),
        trace=trace, **kw)
    # per-core output is [4, DPC//4] with doc = 4*col + row
    out = np.concatenate(
        [res.results[i]["out"].T.reshape(-1) for i in range(N_CORES)])
    return out.astype(np.float32), res


def kernel(q_hidden, d_hidden, W, d_mask, ppq):
    q_hidden = np.asarray(q_hidden, dtype=np.float32)
    d_hidden = np.asarray(d_hidden, dtype=np.float32)
    W = np.asarray(W, dtype=np.float32)
    d_mask = np.asarray(d_mask).astype(bool)
    in_maps = _prep_in_maps(q_hidden, d_hidden, W, d_mask)
    out, _ = _run(in_maps, trace=False)
    return out


# revision 17
# speedup vs baseline: 1.0962x; 1.0252x over previous
"""ColBERT MaxSim kernel for 8 Trainium2 NeuronCores (Bass/Tile).

Strategy (v3): data-parallel over the 256-doc batch (32 docs per core).

Key optimizations over the v1 kernel (75.9us):
- Valid-token compaction on host: the doc mask keeps ~50% of tokens, so
  each doc's valid tokens are gathered into a fixed LV-slot prefix
  (LV = max valid count rounded up to 32; 288 for the spec data) and
  zero-padded.  Cuts HBM traffic and PE/ACT/DVE streaming ~44%.
- Normalize scores instead of vectors: sim is computed on unnormalized
  projections; the per-token 1/|d| scale is applied to the [32q, LV]
  score rows of 4 docs packed in one PSUM bank, so the rsqrt runs once
  per 4 docs instead of on every [128, LV] projection tile.
- The per-token sumsq ones-matmul packs both docs of an even pair into
  one fp8 DoubleRow matmul (zeroed cross-interleave weights give each
  doc its own 32-partition output group; DR can only target partition
  base 0, so odd pairs use per-doc 32-col ones matmuls).
- ss/sim matmuls are emitted one pair behind the projection so the PE
  never stalls on the ACT square / DVE copy of the same pair.
- The first two doc pairs are emitted BEFORE the query stage (and the
  projection PSUM pool sits in different banks than the query pool),
  so doc projections start as soon as the first doc DMA lands instead
  of serializing behind the whole query pipeline.

Per core:
  q_proj = Wt.T @ qT                [128dim, 128q]  -> l2norm -> q_norm
  per pair p (16):
    pd    = Wt8.T @ slab[p]         [128dim, 2, LV] (fp8 DoubleRow, 6 MMs)
    sq    = pd^2                    (ACT, fp8 out)
    dnr   = pd                      (DVE copy, bf16 out)
    ssq[quad]  += ones.T @ sq       (fp8 DR matmul / 32-col matmuls)
    simq[quad] += q32.T @ dnr       (2 bf16 matmuls -> 32 rows each)
  per quad g (8):
    inv  = rsqrt(ssq + eps)         (ACT)
    maxcol[:, g] = max_t(simq*inv)  (DVE mul + reduce_max)
  out[4, 8] = blockones.T @ maxcol  (sum over queries)
"""

import numpy as np
import ml_dtypes

import concourse.bass as bass
import concourse.bass_isa as bass_isa
import concourse.bacc as bacc
import concourse.mybir as mybir
import concourse.tile as tile
from concourse.bass_utils import run_bass_kernel_spmd

N_CORES = 8
H, HC, P = 768, 6, 128   # hidden dim, h-chunks, partitions
LD = 512                 # doc tokens (uncompacted)
DIM = 128                # projection dim
DPC = 32                 # docs per core
QPC = 128                # query vectors per core (4 batches x 32)
PPQ = 8                  # passages per query
NS = 4                   # doc slabs per core
SPD = 8                  # docs per slab
BF16 = mybir.dt.bfloat16
FP8 = mybir.dt.float8e4
F32 = mybir.dt.float32
EPS2 = 1e-12

# pack both docs of an even pair into one fp8 DoubleRow sumsq matmul
USE_DR_SS = True
# fp8 DoubleRow for the sim matmul too (q_norm + projections in fp8)
USE_DR_SIM = False
# fuse normalize-multiply + token max into one DVE tensor_tensor_reduce
# (disabled: crashes the device at runtime despite passing CoreSim)
USE_TTR = False
# pairs of slack between a projection and its ss/sim consumers
DELAY = 1

_NC_CACHE = {}
_CUR_LV = [LD]


def _rsqrt_act(nc, out, in_, bias_ap):
    """out = 1/sqrt(in_ + bias). Emits the Rsqrt activation directly
    (bass's helper refuses it; the 40k-entry reciprocal_sqrt HW table is
    plenty accurate for this kernel's fp8-dominated error budget)."""
    eng = nc.scalar
    ins = [eng.lower_ap(in_), eng.lower_ap(bias_ap),
           mybir.ImmediateValue(dtype=mybir.dt.float32, value=1.0),
           mybir.ImmediateValue(dtype=mybir.dt.float32, value=0.0)]
    return eng.add_instruction(mybir.InstActivation(
        name=nc.get_next_instruction_name(),
        func=mybir.ActivationFunctionType.Rsqrt,
        ins=ins, outs=[eng.lower_ap(out)]))


def _build_nc(LV):
    nc = bacc.Bacc()
    dt_d = nc.declare_dram_parameter("dt", [NS, P, SPD, HC, LV], FP8,
                                     isOutput=False)
    qt_d = nc.declare_dram_parameter("qt", [P, HC, QPC], BF16, isOutput=False)
    wt_d = nc.declare_dram_parameter("wt", [P, HC, DIM], BF16, isOutput=False)
    wt8_d = nc.declare_dram_parameter("wt8", [P, HC, DIM], FP8, isOutput=False)
    out_d = nc.declare_dram_parameter("out", [4, DPC // 4], F32, isOutput=True)

    SQ_DT = FP8 if USE_DR_SS else BF16
    DNR_DT = FP8 if USE_DR_SIM else BF16

    with tile.TileContext(nc) as tc:
        with tc.tile_pool(name="const", bufs=1) as const:
            # first DMAs: what pair-0's projection needs (wt8 + docs 0-1)
            wt8_s = const.tile([P, HC, DIM], FP8)
            nc.sync.dma_start(out=wt8_s, in_=wt8_d[:])

            # Matmul (LDWEIGHTS) instructions only support a single sync
            # wait, so every matmul operand must be produced by a single
            # engine: constants are staged through ACT copies so PE waits
            # coalesce onto one semaphore.
            jones_raw = const.tile([P, P], BF16)
            nc.vector.memset(jones_raw, 1.0)
            jones = const.tile([P, P], BF16)   # all-ones lhsT [K=128, M=128]
            nc.scalar.copy(jones, jones_raw)
            if USE_DR_SS:
                # DR sumsq lhsT [K=128, 2, 64]: cols 0:32 pick interleave 0
                # (doc j=0), cols 32:64 pick interleave 1 (doc j=1)
                odr_raw = const.tile([P, 2, 64], BF16)
                nc.vector.memset(odr_raw, 0.0)
                nc.vector.memset(odr_raw[:, 0, 0:32], 1.0)
                nc.vector.memset(odr_raw[:, 1, 32:64], 1.0)
                ones_dr = const.tile([P, 2, 64], FP8)
                nc.scalar.copy(ones_dr, odr_raw)
                jones8 = const.tile([P, 32], FP8)   # fp8 ones for odd pairs
                nc.scalar.copy(jones8, jones_raw[:, :32])
            blk_raw = const.tile([P, 4], F32)   # block-diag ones: col b = ones
            nc.vector.memset(blk_raw, 0.0)      # on partitions 32b..32b+32
            for b in range(4):
                nc.vector.memset(blk_raw[32 * b:32 * b + 32, b:b + 1], 1.0)
            blockones = const.tile([P, 4], F32)
            nc.scalar.copy(blockones, blk_raw)
            eps_t = const.tile([P, 1], F32)     # rsqrt bias (l2norm eps^2)
            nc.vector.memset(eps_t, EPS2)
            maxcol = const.tile([P, DPC // 4], F32)  # [4docs x 32q, quad-col]
            q_norm = const.tile([DIM, QPC], BF16)
            dummy = const.tile([P, 1], BF16)    # TTR discard target

            wt_s = const.tile([P, HC, DIM], BF16)
            qt_s = const.tile([P, HC, QPC], BF16)

            with (
                tc.tile_pool(name="slabp", bufs=3) as slabp,
                tc.tile_pool(name="work", bufs=3) as work,
                tc.tile_pool(name="psumP", bufs=2,
                             space=bass.MemorySpace.PSUM) as psumP,
            ):
                state = {"slab": None}

                def do_pair_front(pair):
                    """slab DMA + projection + square/copy for one pair."""
                    if pair == 0:
                        # slab 0 fills are staggered (and interleaved with
                        # the wt/qt issues) so pair-0's docs land first and
                        # the PE starts as early as possible
                        slab = slabp.tile([P, SPD, HC, LV], FP8, tag="slab")
                        state["slab"] = slab
                        nc.sync.dma_start(out=slab[:, 0:2], in_=dt_d[0, :, 0:2])
                    elif pair == 1:
                        nc.sync.dma_start(out=state["slab"][:, 2:4],
                                          in_=dt_d[0, :, 2:4])
                    elif pair == 2:
                        nc.sync.dma_start(out=state["slab"][:, 4:8],
                                          in_=dt_d[0, :, 4:8])
                    elif pair % (SPD // 2) == 0:
                        s = pair // (SPD // 2)
                        slab = slabp.tile([P, SPD, HC, LV], FP8, tag="slab")
                        state["slab"] = slab
                        nc.sync.dma_start(out=slab, in_=dt_d[s])
                    slab = state["slab"]
                    r2 = (pair % (SPD // 2)) * 2   # doc-in-slab of doc j=0

                    # projection (fp8 DoubleRow: 256-deep contraction/pass).
                    # doc-major order + per-doc square/cast halves so each
                    # doc's epilogue inputs are ready as early as possible
                    # (the pair-batched versions put the whole ACT square on
                    # the PSUM-rotation critical path).
                    pd = psumP.tile([P, 2, LD], F32, tag="pd")
                    for c in range(0, HC, 2):
                        for j in range(2):
                            nc.tensor.matmul(
                                pd[:, j, :LV], wt8_s[:, c:c + 2, :],
                                slab[:, r2 + j, c:c + 2, :],
                                start=(c == 0), stop=(c == HC - 2),
                                perf_mode=mybir.MatmulPerfMode.DoubleRow)
                    sq = work.tile([P, 2, LV], SQ_DT, tag="sq", bufs=6)
                    nc.scalar.square(sq, pd[:, :, :LV])
                    dnr = work.tile([P, 2, LV], DNR_DT, tag="dnr", bufs=6)
                    nc.vector.tensor_copy(dnr, pd[:, :, :LV])
                    return (pair, sq, dnr)

                # front-run two pairs so doc projections start as soon as
                # the first doc DMA lands (the query stage runs in parallel)
                pending = [do_pair_front(0)]
                nc.sync.dma_start(out=wt_s, in_=wt_d[:])
                nc.sync.dma_start(out=qt_s, in_=qt_d[:])
                pending.append(do_pair_front(1))

                # ---- query projection + L2 normalize ----
                with tc.tile_pool(name="qpsum", bufs=1,
                                  space=bass.MemorySpace.PSUM) as qpsum:
                    psq = qpsum.tile([DIM, QPC], F32, tag="pq")
                    for c in range(HC):
                        nc.tensor.matmul(psq, wt_s[:, c, :], qt_s[:, c, :],
                                         start=(c == 0), stop=(c == HC - 1))
                    sqq = const.tile([DIM, QPC], BF16)
                    nc.scalar.square(sqq, psq)
                    ssqb = qpsum.tile([DIM, QPC], F32, tag="ssq")
                    nc.tensor.matmul(ssqb, jones, sqq, start=True, stop=True)
                    invqb = const.tile([DIM, QPC], F32)
                    _rsqrt_act(nc, invqb, ssqb, eps_t[:, :])
                    nc.vector.tensor_mul(q_norm, psq, invqb)
                    if USE_DR_SIM:
                        # DR sim lhsT per q-batch b: cols 0:32 = q_norm(b) on
                        # interleave 0, cols 32:64 = q_norm(b) on interleave 1
                        qdr_raw = const.tile([P, 4, 2, 64], BF16)
                        nc.vector.memset(qdr_raw, 0.0)
                        for b in range(4):
                            nc.vector.tensor_copy(
                                qdr_raw[:, b, 0, 0:32],
                                q_norm[:, 32 * b:32 * b + 32])
                            nc.vector.tensor_copy(
                                qdr_raw[:, b, 1, 32:64],
                                q_norm[:, 32 * b:32 * b + 32])
                        q_dr = const.tile([P, 4, 2, 64], FP8)
                        nc.scalar.copy(q_dr, qdr_raw)

                # ---- doc loop ----
                with (
                    tc.tile_pool(name="psumS", bufs=2,
                                 space=bass.MemorySpace.PSUM) as psumS,
                    tc.tile_pool(name="psumM", bufs=2,
                                 space=bass.MemorySpace.PSUM) as psumM,
                ):
                    quads = {}     # quad idx -> (ssq, simq)

                    def emit_ss_sim(pair, sq, dnr):
                        """ss/sim matmuls (+ quad epilogue) for a projected
                        pair; emitted DELAY pairs late so the PE never
                        stalls on the same pair's ACT square / DVE copy."""
                        if pair % 2 == 0:
                            quads[pair // 2] = (
                                psumS.tile([P, LD], F32, tag="ss",
                                           name="ssq"),
                                psumM.tile([P, LD], F32, tag="sim",
                                           name="simq"))
                        ssq, simq = quads[pair // 2]
                        d0 = 2 * pair
                        po_base = 32 * (d0 % 4)    # 0 (even pair), 64 (odd)
                        qb = d0 // PPQ             # q-batch (same for a pair)
                        # per-token sumsq -> ssq[po_base : po_base+64]
                        if USE_DR_SS and po_base == 0:
                            nc.tensor.matmul(
                                ssq[po_base:po_base + 64, :LV], ones_dr, sq,
                                start=True, stop=True,
                                perf_mode=mybir.MatmulPerfMode.DoubleRow,
                                tile_position=(0, po_base))
                        else:
                            ssl = jones8 if USE_DR_SS else jones[:, :32]
                            for j in range(2):
                                pb = po_base + 32 * j
                                nc.tensor.matmul(ssq[pb:pb + 32, :LV],
                                                 ssl, sq[:, j, :],
                                                 start=True, stop=True,
                                                 tile_position=(0, pb))
                        # raw sim scores -> simq[po_base : po_base+64]
                        if USE_DR_SIM and po_base == 0:
                            nc.tensor.matmul(
                                simq[po_base:po_base + 64, :LV], q_dr[:, qb],
                                dnr, start=True, stop=True,
                                perf_mode=mybir.MatmulPerfMode.DoubleRow,
                                tile_position=(0, po_base))
                        else:
                            for j in range(2):
                                pb = po_base + 32 * j
                                nc.tensor.matmul(
                                    simq[pb:pb + 32, :LV],
                                    q_norm[:, 32 * qb:32 * qb + 32],
                                    dnr[:, j, :], start=True, stop=True,
                                    tile_position=(0, pb))

                        if pair % 2 == 1:
                            g = pair // 2
                            inv = work.tile([P, LV], F32, tag="inv", bufs=4)
                            _rsqrt_act(nc, inv, ssq[:, :LV], eps_t[:, :])
                            # maxcol[:, g] = max_t(simq * inv)
                            if USE_TTR:
                                nc.vector.tensor_tensor_reduce(
                                    dummy.broadcast_to((P, LV)),
                                    simq[:, :LV], inv,
                                    scale=1.0, scalar=-3.0e38,
                                    op0=mybir.AluOpType.mult,
                                    op1=mybir.AluOpType.max,
                                    accum_out=maxcol[:, g:g + 1])
                            else:
                                simn = work.tile([P, LV], BF16, tag="simn", bufs=4)
                                nc.vector.tensor_mul(simn, simq[:, :LV], inv)
                                nc.vector.reduce_max(out=maxcol[:, g:g + 1],
                                                     in_=simn,
                                                     axis=mybir.AxisListType.X)

                    for pair in range(2, DPC // 2):
                        pending.append(do_pair_front(pair))
                        while len(pending) > DELAY:
                            emit_ss_sim(*pending.pop(0))
                    for args in pending:
                        emit_ss_sim(*args)

                    po = psumS.tile([4, DPC // 4], F32, tag="ss")
                    nc.tensor.matmul(po, blockones, maxcol,
                                     start=True, stop=True)
                    out_s = work.tile([4, DPC // 4], F32, tag="outrow")
                    nc.vector.tensor_copy(out_s, po)
                    nc.sync.dma_start(out=out_d[:], in_=out_s)
    nc.compile()
    return nc


def _get_nc(LV):
    if LV not in _NC_CACHE:
        _NC_CACHE[LV] = _build_nc(LV)
    return _NC_CACHE[LV]


def _prep_in_maps(q_hidden, d_hidden, W, d_mask):
    bf16 = ml_dtypes.bfloat16
    fp8 = ml_dtypes.float8_e4m3

    # Valid-token compaction: gather each doc's unmasked tokens into a
    # fixed LV-slot prefix; padding slots gather zeroed (masked) rows.
    cnt = d_mask.sum(1)
    LV = int(min(LD, max(32, ((int(cnt.max()) + 31) // 32) * 32)))
    _CUR_LV[0] = LV
    dh = d_hidden.astype(fp8)
    dh[~d_mask] = 0
    order = np.argsort(~d_mask, axis=1, kind="stable")   # valid tokens first
    dhc = np.take_along_axis(dh, order[:, :LV, None], axis=1)  # [Bd, LV, H]

    wt_t = np.ascontiguousarray(W.T.reshape(HC, P, DIM).transpose(1, 0, 2))
    wt = wt_t.astype(bf16)
    wt8 = wt_t.astype(fp8)
    in_maps = []
    for c in range(N_CORES):
        dsl = dhc[c * DPC:(c + 1) * DPC]          # [32, LV, 768]
        arr = dsl.reshape(NS, SPD, LV, HC, P)     # (s, j, t, c, p)
        dt = np.ascontiguousarray(arr.transpose(0, 4, 1, 3, 2))
        qsl = q_hidden[c * (DPC // PPQ):(c + 1) * (DPC // PPQ)]
        qm = qsl.reshape(QPC, H).T.reshape(HC, P, QPC)    # [6, 128, 128]
        qt = np.ascontiguousarray(qm.transpose(1, 0, 2)).astype(bf16)
        in_maps.append({"dt": dt, "qt": qt, "wt": wt, "wt8": wt8})
    return in_maps


def _run(in_maps, trace=False, **kw):
    res = run_bass_kernel_spmd(
        _get_nc(_CUR_LV[0]), in_maps, core_ids=list(range(N_CORES)),
        trace=trace, **kw)
    # per-core output is [4, DPC//4] with doc = 4*col + row
    out = np.concatenate(
        [res.results[i]["out"].T.reshape(-1) for i in range(N_CORES)])
    return out.astype(np.float32), res


def kernel(q_hidden, d_hidden, W, d_mask, ppq):
    q_hidden = np.asarray(q_hidden, dtype=np.float32)
    d_hidden = np.asarray(d_hidden, dtype=np.float32)
    W = np.asarray(W, dtype=np.float32)
    d_mask = np.asarray(d_mask).astype(bool)
    in_maps = _prep_in_maps(q_hidden, d_hidden, W, d_mask)
    out, _ = _run(in_maps, trace=False)
    return out
